# revision 1
# baseline (speedup 1.0000x reference)
"""nn_Net_Integral kernel: data-parallel over z_coord across 8 NeuronCores.

Strategy (per sharding hint): shard z_coord (512 -> 8 x 64) across the 8
cores; BSNN params are tiny and replicated. Each core evaluates its own
(64*512, 6) tiled batch for the interior quadrature and the boundary
quadrature (forward + hand-derived VJP), returning a (64, 1) slice of the
output. Host concatenates to (512, 1).

Self-contained: hardcodes NZ=NX=NB=512 and the BSNN architecture.
"""
import numpy as np
import jax
import jax.numpy as jnp
from functools import partial

NZ, NX, NB = 512, 512, 512
N_CORES = 8
ZSH = NZ // N_CORES  # 64 z per core
PI = np.float32(np.pi)


def _block_diag_mask(n_blocks, r, c):
    m = np.zeros((n_blocks * r, n_blocks * c), np.float32)
    for i in range(n_blocks):
        m[i * r:(i + 1) * r, i * c:(i + 1) * c] = 1.0
    return m


_M0 = jnp.asarray(_block_diag_mask(2, 40, 80))    # (80, 160)
_M1 = jnp.asarray(_block_diag_mask(4, 40, 80))    # (160, 320)


def _bsnn(X, Ws, bs):
    X = jnp.sin(X @ Ws[0] + bs[0])
    X = jnp.sin(X @ Ws[1] + bs[1])
    X = jnp.sin(X @ (Ws[2] * _M0) + bs[2])
    X = jnp.sin(X @ (Ws[3] * _M1) + bs[3])
    return X @ Ws[4] + bs[4]


def _f_src(x, case_index):
    c = (case_index + 1).astype(jnp.float32)
    return jnp.sin(PI * c * x[:, 0]) * jnp.sin(PI * x[:, 1]) * jnp.sin(PI * x[:, 2])


def _g_bc(x, btype, case_index):
    c = (case_index + 1).astype(jnp.float32)
    return jnp.sin(c * jnp.sum(x, axis=1)) * (1.0 + 0.1 * btype.astype(x.dtype))


def _a_coef(x):
    return 1.0 + 0.5 * jnp.cos(x[:, 0])


def _tile(x, z):
    nx, nz = x.shape[0], z.shape[0]
    return jnp.concatenate([jnp.tile(x, (nz, 1)), jnp.repeat(z, nx, axis=0)], axis=1)


@partial(jax.jit, static_argnums=())
def _shard_fn(z_sh, xi_coord, xi_wts, xb_coord, xb_wts, xb_normal,
              W0, b0, W1, b1, W2, b2, W3, b3, W4, b4, xb_btype, case_index):
    Ws = [W0, W1, W2, W3, W4]
    bs = [b0, b1, b2, b3, b4]
    nz, nx, nb = z_sh.shape[0], xi_coord.shape[0], xb_coord.shape[0]

    # interior quadrature
    inp_i = _tile(xi_coord, z_sh)                          # (nz*nx, 6)
    G_i = _bsnn(inp_i, Ws, bs).reshape(nz, nx)
    f_i = _f_src(xi_coord, case_index)
    fG_quad = (G_i * f_i[None, :]) @ xi_wts                # (nz,)

    # boundary quadrature via VJP
    inp_b = _tile(xb_coord, z_sh)                          # (nz*nb, 6)
    net = lambda X: _bsnn(X, Ws, bs)
    Gb, vjp = jax.vjp(net, inp_b)
    dG = vjp(jnp.ones_like(Gb))[0][:, :3].reshape(nz, nb, 3)
    Gn = jnp.einsum('znc,nc->zn', dG, xb_normal)
    g_b = _g_bc(xb_coord, xb_btype, case_index)
    a_b = _a_coef(xb_coord)
    gGn_quad = (Gn * (a_b * g_b)[None, :]) @ xb_wts

    return (fG_quad - gGn_quad)[:, None]                   # (nz, 1)


def kernel(**inputs):
    devs = jax.devices()[:N_CORES]

    # normalize inputs to numpy
    z = np.asarray(inputs["z_coord"], np.float32)
    rep_names = ["xi_coord", "xi_wts", "xb_coord", "xb_wts", "xb_normal",
                 "W0", "b0", "W1", "b1", "W2", "b2", "W3", "b3", "W4", "b4"]
    rep = {k: np.asarray(inputs[k], np.float32) for k in rep_names}
    btype = np.asarray(inputs["xb_btype"]).astype(np.int32)
    cidx = np.int32(np.asarray(inputs["case_index"]))

    # place replicated params on each device; shard z
    futures = []
    for d in range(N_CORES):
        dev = devs[d]
        args = [jax.device_put(z[d * ZSH:(d + 1) * ZSH], dev)]
        args += [jax.device_put(rep[k], dev) for k in rep_names]
        args += [jax.device_put(btype, dev), jax.device_put(cidx, dev)]
        futures.append(_shard_fn(*args))

    out = np.concatenate([np.asarray(f) for f in futures], axis=0)
    return out.astype(np.float32)


if __name__ == "__main__":
    # smoke test with random data
    rng = np.random.default_rng(0)
    ins = {
        "xi_coord": rng.random((NX, 3), np.float32),
        "xi_wts": rng.random(NX, np.float32) / NX,
        "xb_coord": rng.random((NB, 3), np.float32),
        "xb_wts": rng.random(NB, np.float32) / NB,
        "xb_normal": rng.standard_normal((NB, 3)).astype(np.float32),
        "z_coord": rng.random((NZ, 3), np.float32),
        "W0": rng.standard_normal((6, 40)).astype(np.float32),
        "b0": rng.standard_normal((1, 40)).astype(np.float32),
        "W1": rng.standard_normal((40, 80)).astype(np.float32),
        "b1": rng.standard_normal((1, 80)).astype(np.float32),
        "W2": rng.standard_normal((80, 160)).astype(np.float32),
        "b2": rng.standard_normal((1, 160)).astype(np.float32),
        "W3": rng.standard_normal((160, 320)).astype(np.float32),
        "b3": rng.standard_normal((1, 320)).astype(np.float32),
        "W4": rng.standard_normal((320, 1)).astype(np.float32),
        "b4": rng.standard_normal((1, 1)).astype(np.float32),
        "xb_btype": rng.integers(0, 3, NB),
        "case_index": 0,
    }
    out = kernel(**ins)
    print("out shape:", out.shape, "dtype:", out.dtype)
    print(out[:4, 0])
